# revision 1
# baseline (speedup 1.0000x reference)
"""Trainium2 Bass kernel for the CGCNN model (8-core SPMD, graph-parallel).

Strategy (per spec sharding hint, adapted):
- Shard graphs (64/core) -> contiguous node ranges via sorted batch vector.
- Within a core, relabel local nodes by descending in-degree; edges live in a
  ragged (block, slot, partition) layout: partition p of block i owns dst node
  i*128+p, slots j=0..Bi-1 hold its incoming edges (padded with a poison table
  row whose f-half is -30000 so sigmoid(f)=0 kills padded messages).
- Per layer: each core computes node projections for its local nodes
  (feat-major h, lhsT-ready), writes the src-projection table slice (bf16),
  AllGather -> global gather table [8*NPAD, 256].
- Edge phase per block: broadcast-prefill G with Pdst block, indirect-DMA
  gather-accumulate src rows into G, per edge-pair: ef matmul (edge_attr^T
  tiles) + identity-matmul of G into PSUM, sigmoid/softplus from PSUM
  (strided), msg = sig*sp, aggregation via paired identity matmuls into PSUM,
  inv-degree scaling, PE transpose back to feat-major, h_conv = aggr + h.
- BatchNorm: masked sums via scalar_tensor_tensor/activation accum_out,
  AllReduce [128,2], fused scale/bias+ReLU via activation, residual add.
- Pooling: per-block matmuls with 1/graph-size one-hot; tiny MLP on device.
Output: [1,64] per core, concatenated on host.
"""
import os
import sys
import numpy as np

sys.path.insert(0, '/opt/trn_rl_repo')
os.environ.setdefault("NEURON_SCRATCHPAD_PAGE_SIZE", "256")

import ml_dtypes

BF16NP = ml_dtypes.bfloat16

N = 50000
E = 1600000
HID = 128
NGRAPH = 512
NCONV = 4
EDGE_DIM = 64
NCORES = 8
GPC = NGRAPH // NCORES
BN_EPS = 1e-5
POISON_VAL = -30000.0

_CACHE = {}


def _host_prep(inputs):
    x = np.asarray(inputs['x'], np.float32)
    ei = np.asarray(inputs['edge_index']).astype(np.int64)
    ea = np.asarray(inputs['edge_attr'], np.float32)
    batch = np.asarray(inputs['batch']).astype(np.int64)
    src, dst = ei[0], ei[1]

    deg = np.bincount(dst, minlength=N)
    node_start = np.searchsorted(batch, np.arange(0, NGRAPH + 1, GPC))
    n_c = np.diff(node_start)
    NPAD = int(np.ceil((n_c.max() + 2) / 128.0) * 128)
    NB = NPAD // 128
    POISON_ROW = NPAD - 1

    percore = []
    rows_of = np.empty(N, np.int64)
    for c in range(NCORES):
        ids = np.arange(node_start[c], node_start[c + 1])
        order = ids[np.argsort(-deg[ids], kind='stable')]
        percore.append(order)
        rows_of[order] = c * NPAD + np.arange(len(order))

    # common per-block max degree
    B = np.zeros((NCORES, NB), np.int64)
    for c in range(NCORES):
        d = np.zeros(NPAD, np.int64)
        d[:n_c[c]] = deg[percore[c]]
        B[c] = d.reshape(NB, 128).max(1)
    Bi = B.max(0)
    Bi = np.maximum(2, ((Bi + 1) // 2) * 2)
    S = int(Bi.sum())
    slot0 = np.concatenate([[0], np.cumsum(Bi)]).astype(np.int64)

    # edge slot assignment (vectorized)
    ro = rows_of[dst]                     # global padded row of dst
    order_e = np.argsort(ro, kind='stable')
    ro_s = ro[order_e]
    src_s = src[order_e]
    e_s = order_e
    # rank within each row
    row_change = np.concatenate([[True], ro_s[1:] != ro_s[:-1]])
    row_first = np.where(row_change)[0]
    starts = np.repeat(row_first, np.diff(np.concatenate([row_first, [len(ro_s)]])))
    rank = np.arange(len(ro_s)) - starts
    c_e = ro_s // NPAD
    r_loc = ro_s % NPAD
    blk = r_loc // 128
    p_e = r_loc % 128
    slot = slot0[blk] + rank
    assert (rank < Bi[blk]).all()

    gidx = np.full((NCORES, 128, S), POISON_ROW, np.int32)  # core0 poison row
    eaT = np.zeros((NCORES, 64, S * 128), BF16NP)
    ea_bf = ea.astype(BF16NP)
    for c in range(NCORES):
        m = c_e == c
        gidx[c, p_e[m], slot[m]] = rows_of[src_s[m]].astype(np.int32)
        cols = slot[m] * 128 + p_e[m]
        eaT[c][:, cols] = ea_bf[e_s[m]].T

    # per-core host tensors
    maps = []
    invc = np.zeros((NCORES, 128, NB), np.float32)
    for c in range(NCORES):
        d = np.zeros(NPAD, np.float32)
        d[:n_c[c]] = np.maximum(deg[percore[c]], 1)
        d[n_c[c]:] = 1.0
        invc[c] = (0.5 / d).reshape(NB, 128).T

    onehot = np.zeros((NCORES, 128, NB * GPC), np.float32)
    mask = np.zeros((NCORES, 128, NPAD), BF16NP)
    xT = np.zeros((NCORES, 9, NPAD), np.float32)
    for c in range(NCORES):
        g_loc = batch[percore[c]] - c * GPC
        gsz = np.bincount(g_loc, minlength=GPC).astype(np.float32)
        oh = np.zeros((NPAD, GPC), np.float32)
        oh[np.arange(n_c[c]), g_loc] = 1.0 / np.maximum(gsz[g_loc], 1.0)
        onehot[c] = oh.reshape(NB, 128, GPC).transpose(1, 0, 2).reshape(128, NB * GPC)
        mask[c, :, :n_c[c]] = 1.0
        xT[c, :, :n_c[c]] = x[percore[c]].T

    meta = dict(NPAD=NPAD, NB=NB, Bi=Bi.tolist(), S=S, slot0=slot0.tolist(),
                POISON_ROW=POISON_ROW)

    # replicated weights
    Wf = np.asarray(inputs['Wf'], np.float32)
    Ws = np.asarray(inputs['Ws'], np.float32)
    bf_ = np.asarray(inputs['bf'], np.float32)
    bs_ = np.asarray(inputs['bs'], np.float32)
    Wsrc = np.concatenate([np.concatenate([Wf[l, HID:2 * HID], Ws[l, HID:2 * HID]], 1)
                           for l in range(NCONV)], 1)          # [128, 4*256]
    Wdst = np.concatenate([np.concatenate([Wf[l, :HID], Ws[l, :HID]], 1)
                           for l in range(NCONV)], 1)          # [128, 4*256]
    Wef = np.concatenate([np.concatenate([Wf[l, 2 * HID:], Ws[l, 2 * HID:]], 1)
                          for l in range(NCONV)], 1).astype(BF16NP)  # [64, 4*256]
    biasfs = np.concatenate([np.concatenate([bf_[l], bs_[l]])[None]
                             for l in range(NCONV)], 1)        # [1, 4*256]
    poison = np.zeros((1, 256), BF16NP)
    poison[0, :HID] = POISON_VAL

    common = dict(
        W_emb=np.asarray(inputs['W_emb'], np.float32),
        bemb_row=np.asarray(inputs['b_emb'], np.float32)[None, :],
        ones1=np.ones((1, 128), np.float32),
        Wsrc=Wsrc, Wdst=Wdst, Wef=Wef, biasfs=biasfs,
        gammaA=np.asarray(inputs['gamma'], np.float32).T.copy(),  # [128, 4]
        betaA=np.asarray(inputs['beta'], np.float32).T.copy(),
        W1=np.asarray(inputs['W1'], np.float32),
        b1=np.asarray(inputs['b1'], np.float32)[:, None],
        W2=np.pad(np.asarray(inputs['W2'], np.float32), ((0, 64), (0, 0))),
        b2=np.asarray(inputs['b2'], np.float32)[:, None],
        I128b=np.eye(128, dtype=BF16NP),
        I128f=np.eye(128, dtype=np.float32),
        poison=poison,
    )
    in_maps = []
    for c in range(NCORES):
        m = dict(common)
        m.update(xT=xT[c], eaT=eaT[c], gidx=gidx[c], invc=invc[c],
                 onehot=onehot[c], maskt=mask[c])
        in_maps.append(m)
    return meta, in_maps, percore, n_c


def _build(meta):
    import concourse.bass as bass
    import concourse.bacc as bacc
    import concourse.tile as tile
    from concourse import mybir

    F32 = mybir.dt.float32
    F16 = mybir.dt.float16
    BF = mybir.dt.bfloat16
    I32 = mybir.dt.int32
    AF = mybir.ActivationFunctionType
    OP = mybir.AluOpType

    NPAD, NB, Bi, S, slot0 = meta['NPAD'], meta['NB'], meta['Bi'], meta['S'], meta['slot0']
    Bmax = max(Bi)
    RG = [list(range(NCORES))]

    nc = bacc.Bacc("TRN2", target_bir_lowering=False, debug=False,
                   num_devices=NCORES)

    def P_(name, shape, dt):
        return nc.declare_dram_parameter(name, shape, dt, isOutput=False)

    xT_d = P_('xT', [9, NPAD], F32)
    eaT_d = P_('eaT', [64, S * 128], BF)
    gidx_d = P_('gidx', [128, S], I32)
    invc_d = P_('invc', [128, NB], F32)
    onehot_d = P_('onehot', [128, NB * GPC], F32)
    mask_d = P_('maskt', [128, NPAD], BF)
    Wemb_d = P_('W_emb', [9, 128], F32)
    bembr_d = P_('bemb_row', [1, 128], F32)
    ones1_d = P_('ones1', [1, 128], F32)
    Wsrc_d = P_('Wsrc', [128, NCONV * 256], F32)
    Wdst_d = P_('Wdst', [128, NCONV * 256], F32)
    Wef_d = P_('Wef', [64, NCONV * 256], BF)
    biasfs_d = P_('biasfs', [1, NCONV * 256], F32)
    gammaA_d = P_('gammaA', [128, NCONV], F32)
    betaA_d = P_('betaA', [128, NCONV], F32)
    W1_d = P_('W1', [128, 64], F32)
    b1_d = P_('b1', [64, 1], F32)
    W2_d = P_('W2', [128, 1], F32)
    b2_d = P_('b2', [1, 1], F32)
    I128b_d = P_('I128b', [128, 128], BF)
    I128f_d = P_('I128f', [128, 128], F32)
    poison_d = P_('poison', [1, 256], BF)
    out_d = nc.declare_dram_parameter('outg', [1, GPC], F32, isOutput=True)

    with tile.TileContext(nc) as tc:
        with tc.tile_pool(name="res", bufs=1) as res, \
             tc.tile_pool(name="gp", bufs=2) as gp, \
             tc.tile_pool(name="wk", bufs=3) as wk, \
             tc.tile_pool(name="ea", bufs=1) as eap, \
             tc.tile_pool(name="ps", bufs=2, space="PSUM") as ps, \
             tc.tile_pool(name="psa", bufs=2, space="PSUM") as psa, \
             tc.tile_pool(name="pst", bufs=2, space="PSUM") as pstp, \
             tc.tile_pool(name="dram", bufs=1, space="DRAM") as dram:

            # ---- resident loads ----
            def load(shape, dt, d, tag):
                t = res.tile(shape, dt, tag=tag)
                nc.sync.dma_start(t[:], d[:])
                return t

            invc_sb = load([128, NB], F32, invc_d, 'invc')
            mask_sb = load([128, NPAD], BF, mask_d, 'mask')
            Wemb_sb = load([9, 128], F32, Wemb_d, 'wemb')
            bembr_sb = load([1, 128], F32, bembr_d, 'bembr')
            ones1_sb = load([1, 128], F32, ones1_d, 'ones1')
            Wsrc_sb = load([128, NCONV * 256], F32, Wsrc_d, 'wsrc')
            Wdst_sb = load([128, NCONV * 256], F32, Wdst_d, 'wdst')
            Wef_sb = load([64, NCONV * 256], BF, Wef_d, 'wef')
            biasfs_sb = load([1, NCONV * 256], F32, biasfs_d, 'biasfs')
            gammaA_sb = load([128, NCONV], F32, gammaA_d, 'gamA')
            betaA_sb = load([128, NCONV], F32, betaA_d, 'betA')
            W1_sb = load([128, 64], F32, W1_d, 'w1')
            b1_sb = load([64, 1], F32, b1_d, 'b1')
            W2_sb = load([128, 1], F32, W2_d, 'w2')
            b2_sb = load([1, 1], F32, b2_d, 'b2')
            I128b_sb = load([128, 128], BF, I128b_d, 'idb')
            I128f_sb = load([128, 128], F32, I128f_d, 'idf')
            poison_sb = load([1, 256], BF, poison_d, 'poi')

            h_loc = res.tile([128, NPAD], F32, tag='hloc')
            h_conv = res.tile([128, NPAD], F32, tag='hconv')
            Pdst_sb = res.tile([128, NB * 256], BF, tag='pdst')
            stats_sb = res.tile([128, 2], F32, tag='stats')
            scrg = res.tile([1, 8], I32, tag='scrg')  # gpsimd wait absorber

            tbl_in = dram.tile([NPAD, 256], BF, tag='tblin')
            tbl_sh = dram.tile([NCORES * NPAD, 256], BF, tag='tblsh')
            stats_in = dram.tile([128, 2], F32, tag='stin')
            stats_out = dram.tile([128, 2], F32, tag='stout')

            # const-AP registry for float biases in activations
            zcol = res.tile([128, 1], F32, tag='zcol')
            nc.vector.memset(zcol[:], 0.0)
            nc.const_aps.aps[(F32, 0.0)] = zcol[:]
            ocol = res.tile([128, 1], F32, tag='ocol')
            nc.vector.memset(ocol[:], 1.0)
            nc.const_aps.aps[(F32, 1.0)] = ocol[:]


            # ---- embed ----
            for t in range(NB):
                xt = wk.tile([9, 128], F32, tag='xt')
                nc.sync.dma_start(xt[:], xT_d[:, t * 128:(t + 1) * 128])
                pe = ps.tile([128, 512], F32, tag='eps')
                nc.tensor.matmul(pe[:, 0:128], lhsT=Wemb_sb[:], rhs=xt[:],
                                 start=True, stop=False)
                nc.tensor.matmul(pe[:, 0:128], lhsT=bembr_sb[:], rhs=ones1_sb[:],
                                 start=False, stop=True)
                nc.vector.scalar_tensor_tensor(
                    out=h_loc[:, t * 128:(t + 1) * 128], in0=pe[:, 0:128],
                    scalar=1.0, in1=mask_sb[:, t * 128:(t + 1) * 128],
                    op0=OP.mult, op1=OP.mult)

            for l in range(NCONV):
                lc = slice(l * 256, (l + 1) * 256)
                # ---- node phase: src table slice + Pdst ----
                for t in range(NB):
                    hsl = h_loc[:, t * 128:(t + 1) * 128]
                    pn = ps.tile([128, 512], F32, tag='eps')
                    nc.tensor.matmul(pn[:, 0:256], lhsT=hsl, rhs=Wsrc_sb[:, lc],
                                     start=True, stop=True)
                    nc.tensor.matmul(pn[:, 256:512], lhsT=hsl, rhs=Wdst_sb[:, lc],
                                     start=True, stop=False)
                    nc.tensor.matmul(pn[:, 256:512], lhsT=ones1_sb[:],
                                     rhs=biasfs_sb[:, lc], start=False, stop=True)
                    st = wk.tile([128, 256], BF, tag='tstage')
                    nc.vector.tensor_copy(st[:], pn[:, 0:256])
                    nc.sync.dma_start(tbl_in[t * 128:(t + 1) * 128, :], st[:])
                    nc.vector.tensor_copy(Pdst_sb[:, t * 256:(t + 1) * 256],
                                          pn[:, 256:512])
                # poison row (pad rows of other cores gather garbage == fine,
                # but padding slots point at core0's poison row; write ours too)
                nc.sync.dma_start(tbl_in[NPAD - 1:NPAD, :], poison_sb[:])
                nc.gpsimd.collective_compute(
                    "AllGather", OP.bypass, replica_groups=RG,
                    ins=[tbl_in.opt()], outs=[tbl_sh.opt()])
                # absorb collective dep: read a few bytes of tbl_sh on gpsimd
                tprobe = wk.tile([1, 128], BF, tag='tprobe')
                nc.gpsimd.dma_start(tprobe[:], tbl_sh[0:1, 0:128])
                nc.gpsimd.tensor_copy(scrg[0:1, 0:1].bitcast(BF),
                                      tprobe[0:1, 0:2])

                # ---- edge phase ----
                for i in range(NB):
                    Bn = Bi[i]
                    s0 = slot0[i]
                    G = gp.tile([128, Bmax * 256], BF, tag='G')
                    Gb = G[:, 0:Bn * 256]
                    gix = wk.tile([128, Bmax], I32, tag='gix')
                    nc.sync.dma_start(gix[:, 0:Bn], gidx_d[:, s0:s0 + Bn])
                    # prefill Pdst broadcast on DVE (bf16 4x)
                    nc.vector.tensor_copy(
                        Gb.rearrange("p (b d) -> p b d", b=Bn),
                        Pdst_sb[:, i * 256:(i + 1) * 256].unsqueeze(1)
                        .to_broadcast([128, Bn, 256]))
                    for s in range(Bn):
                        nc.gpsimd.indirect_dma_start(
                            out=G[:, s * 256:(s + 1) * 256],
                            out_offset=None,
                            in_=tbl_sh[:, :],
                            in_offset=bass.IndirectOffsetOnAxis(
                                ap=gix[:, s:s + 1], axis=0),
                            compute_op=OP.add)
                    ea_blk = eap.tile([64, Bmax * 128], BF, tag='eab')
                    nc.sync.dma_start(ea_blk[:, 0:Bn * 128],
                                      eaT_d[:, s0 * 128:(s0 + Bn) * 128])
                    for j in range(0, Bn, 2):
                        pe = ps.tile([128, 512], F32, tag='eps')
                        nc.tensor.matmul(pe[:, 0:256],
                                         lhsT=ea_blk[:, j * 128:(j + 1) * 128],
                                         rhs=Wef_sb[:, lc], start=True, stop=False)
                        nc.tensor.matmul(pe[:, 256:512],
                                         lhsT=ea_blk[:, (j + 1) * 128:(j + 2) * 128],
                                         rhs=Wef_sb[:, lc], start=True, stop=False)
                        nc.tensor.matmul(pe[:, 0:512], lhsT=I128b_sb[:],
                                         rhs=G[:, j * 256:(j + 2) * 256],
                                         start=False, stop=True)
                        pe4 = pe[:].rearrange("p (j two d) -> p j two d",
                                              j=2, two=2)
                        # th = tanh(f/2) -> G[, j*256 : j*256+256] as fp16
                        # (bf16 would lose ~4e-3 abs near th=-1; the (1+th)
                        # factor cancels catastrophically for negative f)
                        nc.scalar.activation(
                            G[:, j * 256:(j + 1) * 256].bitcast(F16)
                            .rearrange("p (j d) -> p j d", j=2),
                            pe4[:, :, 0, :], AF.Tanh, scale=0.5)
                        # e = exp(s) -> G[, (j+1)*256 : (j+2)*256]
                        nc.scalar.activation(
                            G[:, (j + 1) * 256:(j + 2) * 256]
                            .rearrange("p (j d) -> p j d", j=2),
                            pe4[:, :, 1, :], AF.Exp)
                    npair = Bn // 2
                    th_ap = G[:].bitcast(F16).rearrange(
                        "p (b pair d) -> p b pair d",
                        b=Bmax // 2, pair=2)[:, 0:npair, 0, :]
                    sp_ap = G[:].rearrange("p (b pair d) -> p b pair d",
                                           b=Bmax // 2, pair=2)[:, 0:npair, 1, :]
                    # sp = ln(1 + e)  (one batched call per block)
                    nc.scalar.activation(sp_ap, sp_ap, AF.Ln, bias=1.0)
                    msg_ap = G[:].rearrange(
                        "p (b pair d) -> p b pair d",
                        b=Bmax // 2, pair=2)[:, 0:npair, 0, :]
                    # msg2 = (th + 1) * sp  == 2*sigmoid(f)*softplus(s)
                    nc.vector.scalar_tensor_tensor(
                        out=msg_ap, in0=th_ap, scalar=ocol[:, 0:1], in1=sp_ap,
                        op0=OP.add, op1=OP.mult)
                    pag = psa.tile([128, 256], F32, tag='agg')
                    for j in range(npair):
                        nc.tensor.matmul(pag[:], lhsT=I128b_sb[:],
                                         rhs=G[:, j * 512:j * 512 + 256],
                                         start=(j == 0), stop=(j == npair - 1))
                    agn = wk.tile([128, 128], F32, tag='agn')
                    nc.vector.tensor_scalar_mul(agn[:], pag[:, 0:128],
                                                invc_sb[:, i:i + 1])
                    nc.vector.scalar_tensor_tensor(
                        out=agn[:], in0=pag[:, 128:256],
                        scalar=invc_sb[:, i:i + 1], op0=OP.mult,
                        op1=OP.add, in1=agn[:])
                    ptr = pstp.tile([128, 128], F32, tag='tr')
                    nc.tensor.transpose(ptr[:], agn[:], I128f_sb[:])
                    nc.vector.tensor_tensor(
                        out=h_conv[:, i * 128:(i + 1) * 128], in0=ptr[:],
                        in1=h_loc[:, i * 128:(i + 1) * 128], op=OP.add)

                # ---- batch norm ---- (explicit reduces; squares in dead G)
                nc.vector.scalar_tensor_tensor(
                    out=h_conv[:], in0=h_conv[:], scalar=1.0, in1=mask_sb[:],
                    op0=OP.mult, op1=OP.mult)
                nc.vector.tensor_reduce(stats_sb[:, 0:1], h_conv[:],
                                        axis=mybir.AxisListType.X, op=OP.add)
                sqt = gp.tile([128, Bmax * 256], BF, tag='G')
                sqbuf = sqt[:].bitcast(F32)[:, 0:NPAD]
                nc.vector.scalar_tensor_tensor(
                    out=sqbuf, in0=h_conv[:], scalar=1.0, in1=h_conv[:],
                    op0=OP.mult, op1=OP.mult)
                nc.vector.tensor_reduce(stats_sb[:, 1:2], sqbuf,
                                        axis=mybir.AxisListType.X, op=OP.add)
                nc.sync.dma_start(stats_in[:], stats_sb[:])
                nc.gpsimd.collective_compute(
                    "AllReduce", OP.add, replica_groups=RG,
                    ins=[stats_in.opt()], outs=[stats_out.opt()])
                gst = wk.tile([128, 2], F32, tag='gst')
                nc.sync.dma_start(gst[:], stats_out[:])
                mu = wk.tile([128, 8], F32, tag='mu')
                nc.vector.tensor_scalar_mul(mu[:, 0:1], gst[:, 0:1], 1.0 / N)
                nc.vector.tensor_scalar_mul(mu[:, 1:2], gst[:, 1:2], 1.0 / N)
                # var = ex2 - mu^2
                nc.vector.tensor_tensor(out=mu[:, 2:3], in0=mu[:, 0:1],
                                        in1=mu[:, 0:1], op=OP.mult)
                nc.vector.tensor_tensor(out=mu[:, 3:4], in0=mu[:, 1:2],
                                        in1=mu[:, 2:3], op=OP.subtract)
                nc.vector.tensor_scalar_add(mu[:, 3:4], mu[:, 3:4], BN_EPS)
                # rstd = exp(-0.5*ln(var+eps)); exp/ln tables are accurate
                nc.scalar.activation(mu[:, 4:5], mu[:, 3:4], AF.Ln)
                nc.scalar.activation(mu[:, 5:6], mu[:, 4:5], AF.Exp,
                                     scale=-0.5)
                # A = gamma * rstd ; B = beta - mu * A
                nc.vector.tensor_tensor(out=mu[:, 6:7], in0=gammaA_sb[:, l:l + 1],
                                        in1=mu[:, 5:6], op=OP.mult)
                nc.vector.scalar_tensor_tensor(
                    out=mu[:, 7:8], in0=mu[:, 0:1], scalar=mu[:, 6:7],
                    op0=OP.mult, op1=OP.subtract, in1=betaA_sb[:, l:l + 1])
                nc.vector.tensor_scalar_mul(mu[:, 7:8], mu[:, 7:8], -1.0)
                # h = relu(A*h_conv + B) + h
                nc.scalar.activation(h_conv[:], h_conv[:], AF.Relu,
                                     bias=mu[:, 7:8], scale=mu[:, 6:7])
                nc.vector.tensor_tensor(out=h_loc[:], in0=h_conv[:],
                                        in1=h_loc[:], op=OP.add)

            # ---- pooling + MLP ----
            ppool = psa.tile([128, GPC], F32, tag='pool')
            for i in range(NB):
                ptr = pstp.tile([128, 128], F32, tag='tr')
                nc.tensor.transpose(ptr[:], h_loc[:, i * 128:(i + 1) * 128],
                                    I128f_sb[:])
                hn = wk.tile([128, 128], F32, tag='hn')
                nc.vector.tensor_copy(hn[:], ptr[:])
                oht = wk.tile([128, GPC], F32, tag='oht')
                nc.sync.dma_start(oht[:], onehot_d[:, i * GPC:(i + 1) * GPC])
                nc.tensor.matmul(ppool[:], lhsT=hn[:], rhs=oht[:],
                                 start=(i == 0), stop=(i == NB - 1))
            pooled = wk.tile([128, GPC], F32, tag='pooled')
            nc.vector.tensor_copy(pooled[:], ppool[:])
            pz = ps.tile([128, 512], F32, tag='eps')
            nc.tensor.matmul(pz[0:64, 0:GPC], lhsT=W1_sb[:], rhs=pooled[:],
                             start=True, stop=True)
            z1 = wk.tile([64, GPC], F32, tag='z1')
            nc.scalar.activation(z1[:], pz[0:64, 0:GPC], AF.Exp,
                                 bias=b1_sb[:, 0:1])
            nc.scalar.activation(z1[:], z1[:], AF.Ln, bias=1.0)
            pz2 = pstp.tile([128, 128], F32, tag='tr')
            nc.tensor.matmul(pz2[0:1, 0:GPC], lhsT=W2_sb[0:64, :], rhs=z1[:],
                             start=True, stop=True)
            zo = wk.tile([1, GPC], F32, tag='zo')
            nc.scalar.activation(zo[:], pz2[0:1, 0:GPC], AF.Identity,
                                 bias=b2_sb[0:1, 0:1])
            nc.sync.dma_start(out_d[:], zo[:])

    nc.compile()
    return nc


TRACE = False
LAST_RESULTS = None


def kernel(**inputs):
    global LAST_RESULTS
    from concourse.bass_utils import run_bass_kernel_spmd

    meta, in_maps, percore, n_c = _host_prep(inputs)
    key = (meta['NPAD'], meta['S'], tuple(meta['Bi']))
    if key not in _CACHE:
        _CACHE[key] = _build(meta)
    nc = _CACHE[key]
    res = run_bass_kernel_spmd(nc, in_maps, list(range(NCORES)), trace=TRACE)
    LAST_RESULTS = res
    out = np.concatenate([np.asarray(res.results[c]['outg']).reshape(GPC)
                          for c in range(NCORES)])
    return out.astype(np.float32)


def bench(inputs, reps=8):
    """Steady-state device timing: jit once, inputs device-resident, time
    repeated executes (async-pipelined, block at end)."""
    import time
    import jax
    from jax.sharding import Mesh, PartitionSpec
    from jax.experimental.shard_map import shard_map
    from concourse import bass2jax
    from concourse.bass2jax import _bass_exec_p, partition_id_tensor, \
        install_neuronx_cc_hook
    from concourse import mybir

    meta, in_maps, percore, n_c = _host_prep(inputs)
    key = (meta['NPAD'], meta['S'], tuple(meta['Bi']))
    if key not in _CACHE:
        _CACHE[key] = _build(meta)
    nc = _CACHE[key]
    install_neuronx_cc_hook()
    n_cores = NCORES
    in_names, out_names, out_avals, zero_outs = [], [], [], []
    for alloc in nc.m.functions[0].allocations:
        if not isinstance(alloc, mybir.MemoryLocationSet):
            continue
        name = alloc.memorylocations[0].name
        pn = nc.partition_id_tensor.name if nc.partition_id_tensor else None
        if alloc.kind == "ExternalInput":
            if name != pn:
                in_names.append(name)
        elif alloc.kind == "ExternalOutput":
            out_names.append(name)
            shape = tuple(alloc.tensor_shape)
            dtype = mybir.dt.np(alloc.dtype)
            out_avals.append(jax.core.ShapedArray(shape, dtype))
            zero_outs.append(np.zeros(shape, dtype))
    n_params = len(in_names)
    n_outs = len(out_avals)
    all_names = list(in_names) + out_names
    pn = nc.partition_id_tensor.name if nc.partition_id_tensor else None
    if pn is not None:
        all_names.append(pn)

    def _body(*args):
        operands = list(args)
        if pn is not None:
            operands.append(partition_id_tensor())
        return tuple(_bass_exec_p.bind(
            *operands, out_avals=tuple(out_avals), in_names=tuple(all_names),
            out_names=tuple(out_names), lowering_input_output_aliases=(),
            sim_require_finite=True, sim_require_nnan=True, nc=nc))

    devices = jax.devices()[:n_cores]
    mesh = Mesh(np.asarray(devices), ("core",))
    in_specs = (PartitionSpec("core"),) * (n_params + n_outs)
    out_specs = (PartitionSpec("core"),) * len(out_names)
    sharded = jax.jit(shard_map(_body, mesh=mesh, in_specs=in_specs,
                                out_specs=out_specs, check_rep=False),
                      keep_unused=True)
    concat_in = [np.concatenate([np.asarray(in_maps[c][nm])
                                 for c in range(n_cores)], axis=0)
                 for nm in in_names]
    concat_zeros = [np.zeros((n_cores * z.shape[0], *z.shape[1:]), z.dtype)
                    for z in zero_outs]
    din = [jax.device_put(a) for a in concat_in]
    dzr = [jax.device_put(a) for a in concat_zeros]
    out = sharded(*din, *dzr)  # warmup + compile
    jax.block_until_ready(out)
    t0 = time.time()
    for _ in range(reps):
        out = sharded(*din, *dzr)
    jax.block_until_ready(out)
    dt = (time.time() - t0) / reps
    return dt, out


if __name__ == '__main__':
    import reference as ref
    inputs = {k: np.asarray(v) for k, v in ref.setup_inputs().items()}
    got = kernel(**inputs)
    exp = np.asarray(ref.reference(**ref.setup_inputs()))
    rel = np.abs(got - exp) / np.maximum(np.abs(exp), 1e-6)
    print('rel err max/mean:', rel.max(), rel.mean())

